# revision 15
# baseline (speedup 1.0000x reference)
"""Gated GQA attention block (B=2,S=2048,E=2048,H=16,HKV=2,D=256,RD=64) on 8 TRN2 cores.

Sharding: data-parallel on batch (2 groups of 4 cores); within a group,
tensor-parallel on query heads (4 heads/core). Each core computes its KV head's
k/v projection locally (duplicated across the 2 cores sharing a KV head).
o_proj is row-parallel; the all-reduce over the 4 cores of a group happens on
the host after gather.

All matmuls run bf16 x bf16 (full PE rate); PSUM accumulation is fp32.
Every intermediate (q/g/k/v/gated activations, Wo) is SBUF-resident in bf16 —
no DRAM round trips inside the kernel.

Attention streams exact-width diagonal strips: for a 512-wide q column, the
four kk-tiles overlapping the causal diagonal stream only their valid columns
(512/384/256/128).  PSUM accumulation legality (start/stop flags are per-mm
whole-slice) is preserved by ordering: the first accumulating matmul (one full
tile, or diag strip j0 which is full-width-valid) covers all 512 columns with
start=True, and the last one (diag strip j0, or for qq=0 a mask-zeroed
full-width j3) covers all 512 columns with stop=True.
"""

import sys

if "/opt/trn_rl_repo" not in sys.path:
    sys.path.insert(0, "/opt/trn_rl_repo")

import ml_dtypes
import numpy as np

import concourse.bass as bass
import concourse.tile as tile
from concourse import bacc, mybir
from concourse.bass_utils import run_bass_kernel_spmd

F32 = mybir.dt.float32
BF16 = mybir.dt.bfloat16
AF = mybir.ActivationFunctionType

S = 2048          # tokens per batch element
E = 2048          # model dim
D = 256           # head dim
RD = 64           # rope dims
NHC = 4           # q heads per core
HD = NHC * D      # per-core head dims (1024)
ECH = E // 128    # 16 contraction chunks
QCH = HD // 128   # 8 per-core q/g/o d-chunks
TT = 4            # 512-wide token tiles
NKC = S // 128    # 16 k chunks
NQC = S // 128    # 16 q chunks (oproj)


def _strips(qq):
    """Issue-ordered attention strips for 512-wide q column qq.

    Returns (kk, lo, width, mask, start, stop):
      kk    -- 128-row k tile index
      lo    -- first valid q column (relative to the 512-wide block)
      mask  -- None | 'tri' (mask leading 128 cols) | 'full' (mask all 512)
      start/stop -- PSUM accumulation flags for the av/sm matmuls
    """
    base = 4 * qq
    if qq == 0:
        return [
            (0, 0, 512, "tri", True, False),
            (1, 128, 384, "tri", False, False),
            (2, 256, 256, "tri", False, False),
            (3, 0, 512, "full", False, True),
        ]
    l = [(kk, 0, 512, None, kk == 0, False) for kk in range(base)]
    l += [
        (base + 3, 384, 128, "tri", False, False),
        (base + 2, 256, 256, "tri", False, False),
        (base + 1, 128, 384, "tri", False, False),
        (base + 0, 0, 512, "tri", False, True),
    ]
    return l


def _body(tc, d):
    nc = tc.nc
    ts = bass.ts

    from contextlib import ExitStack

    stack = ExitStack()

    p_big = stack.enter_context(tc.tile_pool(name="big", bufs=1))
    qT = p_big.tile([128, QCH, S], BF16, tag="qT")
    gT = p_big.tile([128, QCH, S], BF16, tag="gT")
    gat = p_big.tile([128, QCH, S], BF16, tag="gat")
    kt = p_big.tile([128, 2, S], BF16, tag="kt")
    vt = p_big.tile([128, NKC, D], BF16, tag="vt")
    mk = p_big.tile([128, 4, 512], BF16, tag="mk")
    ones = p_big.tile([128, 128], BF16, tag="ones")
    rotm = p_big.tile([RD, RD], BF16, tag="rotm")

    psum = stack.enter_context(tc.tile_pool(name="psum", bufs=8, space="PSUM"))

    # ---------------- Phase 1: projections ----------------
    with (
        tc.tile_pool(name="xt", bufs=1) as p_xt,
        tc.tile_pool(name="w", bufs=2) as p_w,
        tc.tile_pool(name="wv", bufs=1) as p_wv,
        tc.tile_pool(name="trig", bufs=1) as p_trig,
        tc.tile_pool(name="rtmp", bufs=1) as p_rtmp,
    ):
        # Queue order matters for the DMA ramp: the PE consumes xt chunks in
        # ec order at ~1.3us/chunk, so x feeds from two queues with nothing
        # queued ahead of it.  scalar: wk0 then wv (first v/k matmul operands),
        # gpsimd: odd xt chunks, then phase-1 constants.
        wkt0 = p_w.tile([128, ECH, 128], BF16, tag="w", name="wkt0")
        for i in range(4):
            nc.scalar.dma_start(wkt0[:, ts(i, 4), :], d["wk"].ap()[0][:, ts(i, 4), :])
        wv_t = p_wv.tile([128, ECH, D], BF16, tag="wv")
        for wh in range(4):
            nc.scalar.dma_start(
                wv_t[:, ts(wh, ECH // 4), :], d["wv"].ap()[:, ts(wh, ECH // 4), :]
            )

        wkt1 = p_w.tile([128, ECH, 128], BF16, tag="w", name="wkt1")
        xt = p_xt.tile([128, ECH, S], BF16, tag="xt")
        qs = [nc.sync, nc.gpsimd, nc.scalar]
        for ec in range(ECH):
            eng = qs[ec % 3]
            if ec < 2:
                for t in range(TT):
                    eng.dma_start(
                        xt[:, ec, ts(t, 512)], d["xt"].ap()[:, ec, ts(t, 512)]
                    )
            else:
                eng.dma_start(xt[:, ec, :], d["xt"].ap()[:, ec, :])
            if ec == 5:
                # wave B's weights, queued so they land well before ~23us
                nc.scalar.dma_start(wkt1[:], d["wk"].ap()[1])

        cos_t = p_trig.tile([RD, S], BF16, tag="cos")
        nc.gpsimd.dma_start(cos_t[:], d["cost"].ap())
        sin_t = p_trig.tile([RD, S], BF16, tag="sin")
        nc.gpsimd.dma_start(sin_t[:], d["sint"].ap())
        nc.gpsimd.dma_start(rotm[:], d["rotm"].ap())
        nc.gpsimd.dma_start(mk[:], d["masks"].ap())
        nc.gpsimd.dma_start(ones[:], d["ones"].ap())

        def rope(dst, t):
            # dst: bf16 SBUF [RD, 512] holding rope dims (partition = d).
            # rot = R @ x via PE, then dst = x*cos + rot*sin.
            rp = psum.tile([RD, 512], F32, tag="ps")
            nc.tensor.matmul(rp[:], rotm[:], dst[:], start=True, stop=True)
            tmp = p_rtmp.tile([RD, 512], F32, tag="rt")
            nc.vector.tensor_mul(tmp[:], dst[:], cos_t[:, ts(t, 512)])
            nc.vector.tensor_mul(dst[:], rp[:], sin_t[:, ts(t, 512)])
            nc.vector.tensor_add(dst[:], dst[:], tmp[:])

        # v + k interleaved ec-outer: their matmuls consume each xt chunk as
        # it streams in, keeping the PE fed through the DMA ramp.
        for wave in range(2):
            wkt = wkt0 if wave == 0 else wkt1
            kps = [psum.tile([128, 512], F32, tag="ps", name=f"kps{wave}_{_i}") for _i in range(4)]
            vps = [psum.tile([128, D], F32, tag="ps", name=f"vps{wave}_{_i}") for _i in range(4)]
            for ec in range(ECH):
                st, en = (ec == 0), (ec == ECH - 1)
                for t in range(TT):
                    nc.tensor.matmul(
                        kps[t][:], wkt[:, ec, :], xt[:, ec, ts(t, 512)],
                        start=st, stop=en,
                    )
                for i in range(4):
                    tcn = 4 * wave + i
                    nc.tensor.matmul(
                        vps[i][:], xt[:, ec, ts(tcn, 128)], wv_t[:, ec, :],
                        start=st, stop=en,
                    )
            for t in range(TT):
                kslice = kt[:, wave, ts(t, 512)]
                nc.scalar.copy(kslice, kps[t][:])
                if wave == 0:
                    rope(kt[0:RD, wave, ts(t, 512)], t)
            for i in range(4):
                nc.scalar.copy(vt[:, 4 * wave + i, :], vps[i][:])

        # remaining v tiles 8..15 (xt fully arrived by now)
        vps = [psum.tile([128, D], F32, tag="ps", name=f"vpsc_{_i}") for _i in range(8)]
        for ec in range(ECH):
            st, en = (ec == 0), (ec == ECH - 1)
            for i in range(8):
                nc.tensor.matmul(
                    vps[i][:], xt[:, ec, ts(8 + i, 128)], wv_t[:, ec, :],
                    start=st, stop=en,
                )
        for i in range(8):
            nc.scalar.copy(vt[:, 8 + i, :], vps[i][:])

        def proj_chunk(w_ap, dstT, idx, kind):
            wt = p_w.tile([128, ECH, 128], BF16, tag="w")
            nc.scalar.dma_start(wt[:], w_ap)
            pss = [psum.tile([128, 512], F32, tag="ps", name=f"pss_{_i}") for _i in range(TT)]
            for ec in range(ECH):
                for t in range(TT):
                    nc.tensor.matmul(
                        pss[t][:], wt[:, ec, :], xt[:, ec, ts(t, 512)],
                        start=(ec == 0), stop=(ec == ECH - 1),
                    )
            for t in range(TT):
                dcols = dstT[:, idx, ts(t, 512)]
                if kind == "g":
                    nc.scalar.activation(dcols, pss[t][:], AF.Sigmoid)
                else:
                    nc.scalar.copy(dcols, pss[t][:])
                    if idx % 2 == 0:
                        rope(dstT[0:RD, idx, ts(t, 512)], t)

        for h in range(NHC):
            for jj in (2 * h, 2 * h + 1):
                proj_chunk(d["wq"].ap()[jj], qT, jj, "q")
            for jj in (2 * h, 2 * h + 1):
                proj_chunk(d["wg"].ap()[jj], gT, jj, "g")

    # ---------------- Phase 2: attention + o_proj ----------------
    with (
        tc.tile_pool(name="wo", bufs=1) as p_wo,
        tc.tile_pool(name="exp", bufs=6) as p_exp,
        tc.tile_pool(name="gtmp", bufs=4) as p_gtmp,
        tc.tile_pool(name="ob", bufs=4) as p_ob,
    ):
        wo_t = p_wo.tile([128, QCH, E], BF16, tag="wo")
        for wh in range(4):
            nc.gpsimd.dma_start(
                wo_t[:, ts(wh, QCH // 4), :], d["wo"].ap()[:, ts(wh, QCH // 4), :]
            )

        def attention(h, qq):
            qcols = ts(qq, 512)
            av0 = psum.tile([128, 512], F32, tag="ps")
            av1 = psum.tile([128, 512], F32, tag="ps")
            smp = psum.tile([128, 512], F32, tag="ps")

            def flush_av(kk, lo, w, ex, st, en):
                nc.tensor.matmul(
                    av0[:, lo : lo + w], vt[:, kk, 0:128], ex, start=st, stop=en
                )
                nc.tensor.matmul(
                    av1[:, lo : lo + w], vt[:, kk, 128:256], ex, start=st, stop=en
                )
                nc.tensor.matmul(
                    smp[:, lo : lo + w], ones[:], ex, start=st, stop=en
                )

            pend = []
            for kk, lo, w, mask, st, en in _strips(qq):
                spp = psum.tile([128, 512], F32, tag="ps")
                sp = spp[:, 0:w]
                qlo = qq * 512 + lo
                nc.tensor.matmul(
                    sp, kt[:, 0, ts(kk, 128)], qT[:, 2 * h, qlo : qlo + w],
                    start=True, stop=False,
                )
                nc.tensor.matmul(
                    sp, kt[:, 1, ts(kk, 128)], qT[:, 2 * h + 1, qlo : qlo + w],
                    start=False, stop=True,
                )
                ext = p_exp.tile([128, 512], BF16, tag="ex")
                ex = ext[:, 0:w]
                nc.scalar.activation(ex, sp, AF.Exp, scale=0.0625)
                if mask == "tri":
                    nc.vector.tensor_mul(ex[:, 0:128], ex[:, 0:128], mk[:, 0, 0:128])
                elif mask == "full":
                    nc.vector.tensor_mul(ex, ex, mk[:, 3, :])
                pend.append((kk, lo, w, ex, st, en))
                if len(pend) > 1:
                    flush_av(*pend.pop(0))
            for p in pend:
                flush_av(*p)

            rec = p_gtmp.tile([128, 512], F32, tag="rec")
            nc.vector.reciprocal_approx_fast(rec[:], smp[:])
            for c, avc in enumerate((av0, av1)):
                g1 = p_gtmp.tile([128, 512], F32, tag="g1")
                nc.vector.tensor_mul(g1[:], avc[:], gT[:, 2 * h + c, qcols])
                nc.vector.tensor_mul(gat[:, 2 * h + c, qcols], g1[:], rec[:])

        def oproj_qc(qc):
            for et in range(4):
                op = psum.tile([128, 512], F32, tag="ps")
                for hc in range(QCH):
                    nc.tensor.matmul(
                        op[:],
                        gat[:, hc, ts(qc, 128)],
                        wo_t[:, hc, ts(et, 512)],
                        start=(hc == 0),
                        stop=(hc == QCH - 1),
                    )
                ob = p_ob.tile([128, 512], F32, tag="ob")
                nc.scalar.copy(ob[:], op[:])
                nc.sync.dma_start(d["out"].ap()[qc][:, ts(et, 512)], ob[:])

        qq_order = [3, 0, 2, 1]  # dense first; sparse columns sit next
        # to dense o_proj work that fills their dependency bubbles
        prev = None
        for qq in qq_order:
            for h in range(NHC):
                if prev is not None:
                    oproj_qc(4 * prev + h)
                attention(h, qq)
            prev = qq
        for i in range(4):
            oproj_qc(4 * prev + i)

    stack.close()


def build_nc():
    nc = bacc.Bacc("TRN2", target_bir_lowering=False, debug=False)
    d = {}
    d["xt"] = nc.dram_tensor("xt", [128, ECH, S], BF16, kind="ExternalInput")
    d["wq"] = nc.dram_tensor("wq", [QCH, 128, ECH, 128], BF16, kind="ExternalInput")
    d["wg"] = nc.dram_tensor("wg", [QCH, 128, ECH, 128], BF16, kind="ExternalInput")
    d["wk"] = nc.dram_tensor("wk", [2, 128, ECH, 128], BF16, kind="ExternalInput")
    d["wv"] = nc.dram_tensor("wv", [128, ECH, D], BF16, kind="ExternalInput")
    d["wo"] = nc.dram_tensor("wo", [128, QCH, E], BF16, kind="ExternalInput")
    d["cost"] = nc.dram_tensor("cost", [RD, S], BF16, kind="ExternalInput")
    d["sint"] = nc.dram_tensor("sint", [RD, S], BF16, kind="ExternalInput")
    d["masks"] = nc.dram_tensor("masks", [128, 4, 512], BF16, kind="ExternalInput")
    d["rotm"] = nc.dram_tensor("rotm", [RD, RD], BF16, kind="ExternalInput")
    d["ones"] = nc.dram_tensor("ones", [128, 128], BF16, kind="ExternalInput")
    d["out"] = nc.dram_tensor("out", [NQC, 128, E], F32, kind="ExternalOutput")
    with tile.TileContext(nc) as tc:
        _body(tc, d)
    nc.compile()
    return nc


_NC_CACHE = None


def _get_nc():
    global _NC_CACHE
    if _NC_CACHE is None:
        _NC_CACHE = build_nc()
    return _NC_CACHE


def _rope_tables():
    inv = 1.0 / (10000.0 ** (np.arange(0, RD, 2, dtype=np.float32) / np.float32(RD)))
    t = np.arange(S, dtype=np.float32)
    freqs = np.outer(t, inv).astype(np.float32)          # [S, RD/2]
    emb = np.concatenate([freqs, freqs], axis=1)         # [S, RD]
    return (
        np.ascontiguousarray(np.cos(emb).astype(np.float32).T),
        np.ascontiguousarray(np.sin(emb).astype(np.float32).T),
    )


def _rotm():
    r = np.zeros((RD, RD), dtype=np.float32)  # r[j, d] = R[d, j], rot = R @ x
    half = RD // 2
    for dd in range(half):
        r[dd + half, dd] = -1.0
    for dd in range(half, RD):
        r[dd - half, dd] = 1.0
    return r


def _masks():
    p = np.arange(128)[:, None, None]
    j = np.arange(4)[None, :, None]
    s = np.arange(512)[None, None, :]
    return ((p + 128 * j) <= s).astype(np.float32)


def _prep_in_maps(hidden_states, Wq, Wk, Wv, Wg, Wo):
    cosT, sinT = _rope_tables()
    masks = _masks().astype(ml_dtypes.bfloat16)
    maps = []
    for c in range(8):
        b, t = c // 4, c % 4
        hq0, kvh = 4 * t, (t // 2)
        cols = slice(hq0 * D, (hq0 + NHC) * D)
        kcols = slice(kvh * D, (kvh + 1) * D)
        x = hidden_states[b]  # [S, E]
        m = {
            "xt": np.ascontiguousarray(
                x.T.reshape(ECH, 128, S).transpose(1, 0, 2)
            ).astype(ml_dtypes.bfloat16),
            "wq": np.ascontiguousarray(
                Wq[:, cols].reshape(ECH, 128, QCH, 128).transpose(2, 1, 0, 3)
            ).astype(ml_dtypes.bfloat16),
            "wg": np.ascontiguousarray(
                Wg[:, cols].reshape(ECH, 128, QCH, 128).transpose(2, 1, 0, 3)
            ).astype(ml_dtypes.bfloat16),
            "wk": np.ascontiguousarray(
                Wk[:, kcols].reshape(ECH, 128, 2, 128).transpose(2, 1, 0, 3)
            ).astype(ml_dtypes.bfloat16),
            "wv": np.ascontiguousarray(
                Wv[:, kcols].reshape(ECH, 128, D).transpose(1, 0, 2)
            ).astype(ml_dtypes.bfloat16),
            "wo": np.ascontiguousarray(
                Wo[cols, :].reshape(QCH, 128, E).transpose(1, 0, 2)
            ).astype(ml_dtypes.bfloat16),
            "cost": cosT.astype(ml_dtypes.bfloat16),
            "sint": sinT.astype(ml_dtypes.bfloat16),
            "masks": masks,
            "rotm": _rotm().astype(ml_dtypes.bfloat16),
            "ones": np.ones((128, 128), dtype=ml_dtypes.bfloat16),
        }
        maps.append(m)
    return maps


def _run(inputs, trace=False, trace_cores=None, tmpdir=None):
    nc = _get_nc()
    in_maps = _prep_in_maps(**inputs)
    kw = {}
    if trace:
        kw = dict(trace=True, trace_cores=trace_cores, tmpdir=tmpdir)
    res = run_bass_kernel_spmd(nc, in_maps, list(range(8)), **kw)
    outs = [res.results[c]["out"].reshape(S, E) for c in range(8)]
    full = np.stack(
        [
            outs[0] + outs[1] + outs[2] + outs[3],
            outs[4] + outs[5] + outs[6] + outs[7],
        ]
    ).astype(np.float32)
    return full, res


def kernel(hidden_states, Wq, Wk, Wv, Wg, Wo):
    full, _ = _run(
        dict(hidden_states=np.asarray(hidden_states, dtype=np.float32),
             Wq=np.asarray(Wq, dtype=np.float32),
             Wk=np.asarray(Wk, dtype=np.float32),
             Wv=np.asarray(Wv, dtype=np.float32),
             Wg=np.asarray(Wg, dtype=np.float32),
             Wo=np.asarray(Wo, dtype=np.float32))
    )
    return full
